# revision 21
# baseline (speedup 1.0000x reference)
"""DeepSeekV3 router kernel for Trainium2 (8 NeuronCores, data-parallel over tokens).

Computes, for x[T,D] @ W[D,E] -> sigmoid -> biased grouped top-k routing:
  weights[T,8] (normalized, scaled) and indices[T,8] (int32).

Sharding: x split along T across 8 cores; W and bias replicated.

Design: the fp16 hi/lo split of x and W AND the transpose of x happen on
the host (numerically identical to doing the split on-chip: the same
fp32->fp16 round-to-nearest and exact fp32 subtract).  Each core receives
xT hi/lo pre-arranged as [n_groups, 128(d-part), 56(k-chunk), group_tokens]
fp16 in a fat-DMA-friendly layout, so the device does nothing but:

  - stream pure-fp16 matmuls on the PE (3-term hi/lo product, exact to
    ~2^-22: z = xh.wh + xl.wh + xh.wl*(1/1024), wl prescaled by 1024),
    accumulating z^T per 128-expert half in PSUM (one accumulation series
    per 2KB PSUM bank — interleaved series in one bank corrupt each other),
  - drain z^T + combine, transpose 128x128 blocks back on the PE (the only
    fp32 matmuls left, ~20 total), sigmoid on ACT,
  - hierarchical top-k routing on DVE (max/max_index/match_replace + an
    8x8 broadcast-AP permutation-match to recover weights without a gather).

Pipelining: 256-token groups with double-buffered PSUM accumulators and x
tiles; DMAs issued in consumption order on one queue (the ~8-slot HWDGE
window fair-shares ring bandwidth, so issue order = arrival order); the
final group is split into two 128-token segments so its routing tail
overlaps the last matmuls.  Measured ~183us on 8 trn2 cores (baseline
this session started from: ~297us; PE matmul floor for the 3-pass scheme
is ~143us/core + ~15us of fixed framework pre/postamble).
"""

import os
import numpy as np

import bass_rust
import concourse.bacc as bacc
import concourse.bass as bass
import concourse.mybir as mybir
from concourse import tile, masks
from concourse import bass_utils

F32 = mybir.dt.float32
F16 = mybir.dt.float16
U32 = mybir.dt.uint32
I32 = mybir.dt.int32
ALU = mybir.AluOpType
ACTF = mybir.ActivationFunctionType

# Problem constants (hardcoded per contest rules)
T_FULL, D_FULL, E = 8192, 7168, 256
N_CORES = 8
N_GROUPS, TOPK_GROUPS, TOP_K = 8, 4, 8
EPG = E // N_GROUPS  # 32 experts per group
SCALE = 2.5
WL_SCALE = 1024.0  # keeps the W residual in fp16 normal range

T_CORE = T_FULL // N_CORES  # 1024
NK = D_FULL // 128  # 56 contraction chunks
GT = int(os.environ.get("DSV3_GT", "256"))  # tokens per matmul group
NG = T_CORE // GT  # groups per core
NT_G = GT // 128  # 128-token routing tiles per group


def build(tc: tile.TileContext, aps: dict):
    nc = tc.nc
    xh_d, xl_d = aps["xh"], aps["xl"]
    wh_d, wl_d, b_d = aps["wh"], aps["wl"], aps["b"]
    wout_d, iout_d = aps["w_out"], aps["i_out"]

    from contextlib import ExitStack

    ctx = ExitStack()
    const = ctx.enter_context(tc.tile_pool(name="const", bufs=1))
    x_pool = ctx.enter_context(
        tc.tile_pool(name="x", bufs=int(os.environ.get("DSV3_XBUFS", "2")))
    )
    z_pool = ctx.enter_context(tc.tile_pool(name="z", bufs=int(os.environ.get("DSV3_ZBUFS","1")), space="PSUM"))
    zf_pool = ctx.enter_context(tc.tile_pool(name="zf", bufs=2, space="PSUM"))
    ztsb_pool = ctx.enter_context(tc.tile_pool(name="ztsb", bufs=2))
    r_pool = ctx.enter_context(tc.tile_pool(name="r", bufs=2))
    sm_pool = ctx.enter_context(tc.tile_pool(name="small", bufs=2))

    # ---- constants ----
    wh = const.tile([128, NK, E], F16, tag="wh")
    wl = const.tile([128, NK, E], F16, tag="wl")
    bias_sb = const.tile([128, E], F32, tag="bias")
    ident = const.tile([128, 128], F32, tag="ident")

    XCH = 14  # k-chunks per x sub-DMA

    def emit_x_dma(g):
        xh_g = x_pool.tile([128, NK, GT], F16, tag="xh", name=f"xh_g{g}")
        xl_g = x_pool.tile([128, NK, GT], F16, tag="xl", name=f"xl_g{g}")
        for c0 in range(0, NK, XCH):
            nc.sync.dma_start(xh_g[:, c0 : c0 + XCH, :], xh_d[g, :, c0 : c0 + XCH, :])
            nc.sync.dma_start(xl_g[:, c0 : c0 + XCH, :], xl_d[g, :, c0 : c0 + XCH, :])
        return xh_g, xl_g

    # group-0 x and W interleaved in consumption order on one queue: the
    # HWDGE keeps ~8 DMAs in flight and the rings fair-share bandwidth over
    # all of them, so keeping the in-flight window aligned with consumption
    # order matters more than queue parallelism (a dual-queue split measured
    # slower).  The first chunks are small (7 k-chunks) so the first
    # matmul's three dependencies arrive fast.
    xh_g0 = x_pool.tile([128, NK, GT], F16, tag="xh", name="xh_g0")
    xl_g0 = x_pool.tile([128, NK, GT], F16, tag="xl", name="xl_g0")
    for c0, c1 in ((0, 7), (7, 14), (14, 28), (28, 42), (42, 56)):
        nc.sync.dma_start(wh[:, c0:c1, :], wh_d[:, c0:c1, :])
        nc.sync.dma_start(wl[:, c0:c1, :], wl_d[:, c0:c1, :])
        nc.sync.dma_start(xh_g0[:, c0:c1, :], xh_d[0, :, c0:c1, :])
        nc.sync.dma_start(xl_g0[:, c0:c1, :], xl_d[0, :, c0:c1, :])
    nc.scalar.dma_start(bias_sb, b_d[None, :].broadcast_to([128, E]))
    masks.make_identity(nc, ident)
    xtiles = {0: (xh_g0, xl_g0)}

    # The final 256-token group is split into two 128-token segments so the
    # second-to-last segment's routing overlaps the last segment's matmuls,
    # halving the serialized DVE routing tail.
    segs = [(g, 0, GT) for g in range(NG - 1)]
    segs += [(NG - 1, 0, GT // 2), (NG - 1, GT // 2, GT - GT // 2)]

    for si, (g, off, gt) in enumerate(segs):
        if off == 0:
            if g + 1 < NG:
                xtiles[g + 1] = emit_x_dma(g + 1)
        xh_g, xl_g = xtiles[g]
        if off + gt == GT:
            xtiles.pop(g)

        # z^T accumulators: [128e(half), 2 halves, gt tokens] in PSUM.
        # Each half's accumulation series must own a full 2KB PSUM bank
        # (two interleaved start/stop series in one bank corrupt each other),
        # so pad the token dim to 512 f32 = one bank per half.
        PB = 2048 // 4  # fp32 elems per bank
        zm = z_pool.tile([128, 2, gt], F32, tag="zm", name=f"zm_s{si}",
                         padded_shape=[128, 2, PB], bufs=2)
        zw = z_pool.tile([128, 2, gt], F32, tag="zw", name=f"zw_s{si}",
                         padded_shape=[128, 2, PB], bufs=1)
        ts = slice(off, off + gt)
        for kk in range(NK):
            first, last = kk == 0, kk == NK - 1
            for h in range(2):
                hs = slice(h * 128, (h + 1) * 128)
                nc.tensor.matmul(zm[:, h, :], wh[:, kk, hs], xh_g[:, kk, ts],
                                 start=first, stop=False)
                nc.tensor.matmul(zm[:, h, :], wh[:, kk, hs], xl_g[:, kk, ts],
                                 start=False, stop=last)
                nc.tensor.matmul(zw[:, h, :], wl[:, kk, hs], xh_g[:, kk, ts],
                                 start=first, stop=last)

        # drain z^T = zm + zw/WL_SCALE to SBUF, then per-token-tile routing
        ztsb = ztsb_pool.tile([128, 2, gt], F32, tag="ztsb", name=f"ztsb_s{si}",
                              padded_shape=[128, 2, GT])
        for h in range(2):
            nc.scalar.copy(ztsb[:, h, :], zm[:, h, :])
            nc.vector.scalar_tensor_tensor(
                ztsb[:, h, :], zw[:, h, :], 1.0 / WL_SCALE, ztsb[:, h, :],
                op0=ALU.mult, op1=ALU.add,
            )
        for j in range(gt // 128):
            t0 = g * GT + off + j * 128
            _routing_tile(
                nc, tc, ztsb, j, t0, bias_sb, ident, zf_pool, r_pool, sm_pool,
                wout_d, iout_d,
            )

    ctx.close()


def _routing_tile(
    nc, tc, ztsb, j, t0, bias_sb, ident, zf_pool, r_pool, sm_pool, wout_d, iout_d
):
    # transpose z^T block back to [tok, e] and apply sigmoid
    zf = zf_pool.tile([128, 2, 128], F32, tag="zf")
    scores = r_pool.tile([128, E], F32, tag="scores")
    for h in range(2):
        nc.tensor.transpose(zf[:, h, :], ztsb[:, h, j * 128 : (j + 1) * 128], ident)
        nc.scalar.activation(
            scores[:, h * 128 : (h + 1) * 128], zf[:, h, :], ACTF.Sigmoid
        )

    # s = scores + bias (selection key)
    s = r_pool.tile([128, E], F32, tag="s")
    nc.vector.tensor_tensor(s, scores, bias_sb, op=ALU.add)

    # group scores: sum of top-2 within each group of 32
    gtop = sm_pool.tile([128, N_GROUPS, 8], F32, tag="gtop")
    for grp in range(N_GROUPS):
        nc.vector.max(gtop[:, grp, :], s[:, grp * EPG : (grp + 1) * EPG])
    gscore = sm_pool.tile([128, N_GROUPS], F32, tag="gscore")
    nc.vector.tensor_tensor(gscore, gtop[:, :, 0], gtop[:, :, 1], op=ALU.add)

    # top-4 groups: sort the 8 group scores, threshold at the 4th
    gsort = sm_pool.tile([128, 8], F32, tag="gsort")
    nc.vector.max(gsort, gscore)
    keep = sm_pool.tile([128, N_GROUPS], F32, tag="keep")
    nc.vector.tensor_scalar(
        keep, gscore, gsort[:, TOPK_GROUPS - 1 : TOPK_GROUPS], None, op0=ALU.is_ge
    )

    # masked selection key: s * keep (keep broadcast over each group of 32)
    sm_t = r_pool.tile([128, E], F32, tag="smask")
    nc.vector.tensor_tensor(
        sm_t.rearrange("p (g e) -> p g e", g=N_GROUPS),
        s.rearrange("p (g e) -> p g e", g=N_GROUPS),
        keep[:, :, None].broadcast_to([128, N_GROUPS, EPG]),
        op=ALU.mult,
    )

    # top-8 experts by masked biased score
    v8 = sm_pool.tile([128, 8], F32, tag="v8")
    nc.vector.max(v8, sm_t)
    idx8 = sm_pool.tile([128, 8], U32, tag="idx8")
    nc.vector.max_index(idx8, v8, sm_t)

    # indicator of the selected 8 positions (kill them, then compare)
    srest = r_pool.tile([128, E], F32, tag="srest")
    nc.vector.match_replace(
        out=srest, in_to_replace=v8, in_values=sm_t, imm_value=-1e30
    )
    ind = r_pool.tile([128, E], F32, tag="ind")
    nc.vector.tensor_scalar(ind, srest, -1e29, None, op0=ALU.is_le)

    # selected raw scores (zeros elsewhere)
    scsel = r_pool.tile([128, E], F32, tag="scsel")
    nc.vector.tensor_tensor(scsel, scores, ind, op=ALU.mult)

    # the 8 selected scores, sorted by score (order differs from idx8's order)
    s8 = sm_pool.tile([128, 8], F32, tag="s8")
    nc.vector.max(s8, scsel)
    sidx8 = sm_pool.tile([128, 8], U32, tag="sidx8")
    nc.vector.max_index(sidx8, s8, scsel)

    # sum of the selected scores (+eps)
    sumw = sm_pool.tile([128, 1], F32, tag="sumw")
    nc.vector.reduce_sum(sumw, s8, axis=bass_rust.AxisListType.X)
    nc.vector.tensor_scalar_add(sumw, sumw, 1e-20)

    # reorder s8 into idx8's (selection) order via an 8x8 match matrix:
    # w[k] = sum_j (idx8[k]==sidx8[j]) * s8[j]
    idx8f = sm_pool.tile([128, 8], F32, tag="idx8f")
    nc.vector.tensor_copy(idx8f, idx8)
    sidx8f = sm_pool.tile([128, 8], F32, tag="sidx8f")
    nc.vector.tensor_copy(sidx8f, sidx8)
    eq = sm_pool.tile([128, 8, 8], F32, tag="eq")
    nc.vector.tensor_tensor(
        eq,
        idx8f[:, :, None].broadcast_to([128, 8, 8]),
        sidx8f[:, None, :].broadcast_to([128, 8, 8]),
        op=ALU.is_equal,
    )
    nc.vector.tensor_tensor(
        eq, eq, s8[:, None, :].broadcast_to([128, 8, 8]), op=ALU.mult
    )
    wacc = sm_pool.tile([128, 8], F32, tag="wacc")
    nc.vector.tensor_reduce(wacc, eq, axis=bass_rust.AxisListType.X, op=ALU.add)

    # normalize + scale
    winv = sm_pool.tile([128, 1], F32, tag="winv")
    nc.vector.reciprocal(winv, sumw)
    wout = sm_pool.tile([128, 8], F32, tag="wout")
    nc.vector.tensor_scalar(wout, wacc, winv[:, 0:1], SCALE, op0=ALU.mult, op1=ALU.mult)
    iout = sm_pool.tile([128, 8], I32, tag="iout")
    nc.vector.tensor_copy(iout, idx8)

    nc.scalar.dma_start(wout_d[t0 : t0 + 128, :], wout)
    nc.scalar.dma_start(iout_d[t0 : t0 + 128, :], iout)


def make_nc():
    nc = bacc.Bacc(
        "TRN2",
        target_bir_lowering=False,
        debug=False,
        enable_asserts=False,
        num_devices=N_CORES,
    )
    aps = {
        "xh": nc.dram_tensor("xh", [NG, 128, NK, GT], F16, kind="ExternalInput").ap(),
        "xl": nc.dram_tensor("xl", [NG, 128, NK, GT], F16, kind="ExternalInput").ap(),
        "wh": nc.dram_tensor("wh", [128, NK, E], F16, kind="ExternalInput").ap(),
        "wl": nc.dram_tensor("wl", [128, NK, E], F16, kind="ExternalInput").ap(),
        "b": nc.dram_tensor("b", [E], F32, kind="ExternalInput").ap(),
        "w_out": nc.dram_tensor(
            "w_out", [T_CORE, TOP_K], F32, kind="ExternalOutput"
        ).ap(),
        "i_out": nc.dram_tensor(
            "i_out", [T_CORE, TOP_K], I32, kind="ExternalOutput"
        ).ap(),
    }
    with tile.TileContext(nc) as tc:
        build(tc, aps)
    nc.compile()
    return nc


_CACHED = {}


def _get_nc():
    if "nc" not in _CACHED:
        _CACHED["nc"] = make_nc()
    return _CACHED["nc"]


def _split_f16(a32):
    """fp16 hi/lo split, identical rounding to the on-chip ACT copy + DVE
    subtract the v1 kernel used: hi = rne_f16(a); lo = rne_f16(a - hi)."""
    hi = a32.astype(np.float16)
    lo = (a32 - hi.astype(np.float32)).astype(np.float16)
    return hi, lo


def kernel(x_TD, kernel_DE, bias_E, profile=False, trace_kwargs=None):
    x_TD = np.asarray(x_TD, dtype=np.float32)
    kernel_DE = np.asarray(kernel_DE, dtype=np.float32)
    bias_E = np.ascontiguousarray(np.asarray(bias_E, dtype=np.float32))
    assert x_TD.shape == (T_FULL, D_FULL)

    # host-side hi/lo split + transpose into the DMA-friendly layout
    xh, xl = _split_f16(x_TD)
    wh = kernel_DE.astype(np.float16)
    wl = ((kernel_DE - wh.astype(np.float32)) * WL_SCALE).astype(np.float16)

    # x: [T, D] -> per core [NG, 128(p), NK(c), GT(t)];  tok = g*GT+t, d = c*128+p
    def xr(a):
        a = a.reshape(N_CORES, NG, GT, NK, 128)
        return np.ascontiguousarray(a.transpose(0, 1, 4, 3, 2))

    xh_r = xr(xh)
    xl_r = xr(xl)
    # W: [D, E] -> [128(p), NK(c), E]
    wh_r = np.ascontiguousarray(wh.reshape(NK, 128, E).transpose(1, 0, 2))
    wl_r = np.ascontiguousarray(wl.reshape(NK, 128, E).transpose(1, 0, 2))

    nc = _get_nc()
    in_maps = [
        {
            "xh": xh_r[i],
            "xl": xl_r[i],
            "wh": wh_r,
            "wl": wl_r,
            "b": bias_E,
        }
        for i in range(N_CORES)
    ]
    res = bass_utils.run_bass_kernel_spmd(
        nc,
        in_maps,
        core_ids=list(range(N_CORES)),
        trace=profile,
        **(trace_kwargs or {}),
    )
    w_full = np.concatenate([res.results[i]["w_out"] for i in range(N_CORES)], axis=0)
    i_full = np.concatenate([res.results[i]["i_out"] for i in range(N_CORES)], axis=0)
    i_full = i_full.astype(np.int32)
    if profile:
        return (w_full, i_full), res
    return w_full, i_full


# revision 24
# speedup vs baseline: 1.0010x; 1.0010x over previous
"""DeepSeekV3 router kernel for Trainium2 (8 NeuronCores, data-parallel over tokens).

Computes, for x[T,D] @ W[D,E] -> sigmoid -> biased grouped top-k routing:
  weights[T,8] (normalized, scaled) and indices[T,8] (int32).

Sharding: x split along T across 8 cores; W and bias replicated.

Design: the fp16 hi/lo split of x and W AND the transpose of x happen on
the host (numerically identical to doing the split on-chip: the same
fp32->fp16 round-to-nearest and exact fp32 subtract).  Each core receives
xT hi/lo pre-arranged as [n_groups, 128(d-part), 56(k-chunk), group_tokens]
fp16 in a fat-DMA-friendly layout, so the device does nothing but:

  - stream pure-fp16 matmuls on the PE (3-term hi/lo product, exact to
    ~2^-22: z = xh.wh + xl.wh + xh.wl*(1/1024), wl prescaled by 1024),
    accumulating z^T per 128-expert half in PSUM (one accumulation series
    per 2KB PSUM bank — interleaved series in one bank corrupt each other),
  - drain z^T + combine, transpose 128x128 blocks back on the PE (the only
    fp32 matmuls left, ~20 total), sigmoid on ACT,
  - hierarchical top-k routing on DVE (max/max_index/match_replace + an
    8x8 broadcast-AP permutation-match to recover weights without a gather).

Pipelining: 256-token groups with double-buffered PSUM accumulators and x
tiles; DMAs issued in consumption order on one queue (the ~8-slot HWDGE
window fair-shares ring bandwidth, so issue order = arrival order); the
final group is split into two 128-token segments so its routing tail
overlaps the last matmuls.  Measured ~183us on 8 trn2 cores (baseline
this session started from: ~297us; PE matmul floor for the 3-pass scheme
is ~143us/core + ~15us of fixed framework pre/postamble).
"""

import os
import numpy as np

import bass_rust
import concourse.bacc as bacc
import concourse.bass as bass
import concourse.mybir as mybir
from concourse import tile, masks
from concourse import bass_utils

F32 = mybir.dt.float32
F16 = mybir.dt.float16
U32 = mybir.dt.uint32
I32 = mybir.dt.int32
ALU = mybir.AluOpType
ACTF = mybir.ActivationFunctionType

# Problem constants (hardcoded per contest rules)
T_FULL, D_FULL, E = 8192, 7168, 256
N_CORES = 8
N_GROUPS, TOPK_GROUPS, TOP_K = 8, 4, 8
EPG = E // N_GROUPS  # 32 experts per group
SCALE = 2.5
WL_SCALE = 1024.0  # keeps the W residual in fp16 normal range

T_CORE = T_FULL // N_CORES  # 1024
NK = D_FULL // 128  # 56 contraction chunks
GT = int(os.environ.get("DSV3_GT", "256"))  # tokens per matmul group
NG = T_CORE // GT  # groups per core
NT_G = GT // 128  # 128-token routing tiles per group


def build(tc: tile.TileContext, aps: dict):
    nc = tc.nc
    xh_d, xl_d = aps["xh"], aps["xl"]
    wh_d, wl_d, b_d = aps["wh"], aps["wl"], aps["b"]
    wout_d, iout_d = aps["w_out"], aps["i_out"]

    from contextlib import ExitStack

    ctx = ExitStack()
    const = ctx.enter_context(tc.tile_pool(name="const", bufs=1))
    x_pool = ctx.enter_context(
        tc.tile_pool(name="x", bufs=int(os.environ.get("DSV3_XBUFS", "2")))
    )
    z_pool = ctx.enter_context(tc.tile_pool(name="z", bufs=int(os.environ.get("DSV3_ZBUFS","1")), space="PSUM"))
    zf_pool = ctx.enter_context(tc.tile_pool(name="zf", bufs=2, space="PSUM"))
    ztsb_pool = ctx.enter_context(tc.tile_pool(name="ztsb", bufs=2))
    r_pool = ctx.enter_context(tc.tile_pool(name="r", bufs=2))
    sm_pool = ctx.enter_context(tc.tile_pool(name="small", bufs=2))

    # ---- constants ----
    wh = const.tile([128, NK, E], F16, tag="wh")
    wl = const.tile([128, NK, E], F16, tag="wl")
    bias_sb = const.tile([128, E], F32, tag="bias")
    ident = const.tile([128, 128], F32, tag="ident")

    XCH = 14  # k-chunks per x sub-DMA

    def emit_x_dma(g):
        xh_g = x_pool.tile([128, NK, GT], F16, tag="xh", name=f"xh_g{g}")
        xl_g = x_pool.tile([128, NK, GT], F16, tag="xl", name=f"xl_g{g}")
        for c0 in range(0, NK, XCH):
            nc.sync.dma_start(xh_g[:, c0 : c0 + XCH, :], xh_d[g, :, c0 : c0 + XCH, :])
            nc.sync.dma_start(xl_g[:, c0 : c0 + XCH, :], xl_d[g, :, c0 : c0 + XCH, :])
        return xh_g, xl_g

    # group-0 x and W interleaved in consumption order on one queue: the
    # HWDGE keeps ~8 DMAs in flight and the rings fair-share bandwidth over
    # all of them, so keeping the in-flight window aligned with consumption
    # order matters more than queue parallelism (a dual-queue split measured
    # slower).  The first chunks are small (7 k-chunks) so the first
    # matmul's three dependencies arrive fast.
    xh_g0 = x_pool.tile([128, NK, GT], F16, tag="xh", name="xh_g0")
    xl_g0 = x_pool.tile([128, NK, GT], F16, tag="xl", name="xl_g0")
    for c0, c1 in ((0, 7), (7, 14), (14, 28), (28, 42), (42, 56)):
        nc.sync.dma_start(wh[:, c0:c1, :], wh_d[:, c0:c1, :])
        nc.sync.dma_start(wl[:, c0:c1, :], wl_d[:, c0:c1, :])
        nc.sync.dma_start(xh_g0[:, c0:c1, :], xh_d[0, :, c0:c1, :])
        nc.sync.dma_start(xl_g0[:, c0:c1, :], xl_d[0, :, c0:c1, :])
    nc.scalar.dma_start(bias_sb, b_d[None, :].broadcast_to([128, E]))
    masks.make_identity(nc, ident)
    xtiles = {0: (xh_g0, xl_g0)}

    # The final 256-token group is split into two 128-token segments so the
    # second-to-last segment's routing overlaps the last segment's matmuls,
    # halving the serialized DVE routing tail.
    segs = [(g, 0, GT) for g in range(NG - 1)]
    segs += [(NG - 1, 0, GT // 2), (NG - 1, GT // 2, GT - GT // 2)]

    for si, (g, off, gt) in enumerate(segs):
        if off == 0:
            if g + 1 < NG:
                xtiles[g + 1] = emit_x_dma(g + 1)
        xh_g, xl_g = xtiles[g]
        if off + gt == GT:
            xtiles.pop(g)

        # z^T accumulators: [128e(half), 2 halves, gt tokens] in PSUM.
        # Each half's accumulation series must own a full 2KB PSUM bank
        # (two interleaved start/stop series in one bank corrupt each other),
        # so pad the token dim to 512 f32 = one bank per half.
        PB = 2048 // 4  # fp32 elems per bank
        zm = z_pool.tile([128, 2, gt], F32, tag="zm", name=f"zm_s{si}",
                         padded_shape=[128, 2, PB], bufs=2)
        zw = z_pool.tile([128, 2, gt], F32, tag="zw", name=f"zw_s{si}",
                         padded_shape=[128, 2, PB], bufs=1)
        ts = slice(off, off + gt)
        for kk in range(NK):
            first, last = kk == 0, kk == NK - 1
            for h in range(2):
                hs = slice(h * 128, (h + 1) * 128)
                nc.tensor.matmul(zm[:, h, :], wh[:, kk, hs], xh_g[:, kk, ts],
                                 start=first, stop=False)
                nc.tensor.matmul(zm[:, h, :], wh[:, kk, hs], xl_g[:, kk, ts],
                                 start=False, stop=last)
                nc.tensor.matmul(zw[:, h, :], wl[:, kk, hs], xh_g[:, kk, ts],
                                 start=first, stop=last)

        # drain z^T = zm + zw/WL_SCALE to SBUF, then per-token-tile routing
        ztsb = ztsb_pool.tile([128, 2, gt], F32, tag="ztsb", name=f"ztsb_s{si}",
                              padded_shape=[128, 2, GT])
        for h in range(2):
            nc.scalar.copy(ztsb[:, h, :], zm[:, h, :])
            nc.vector.scalar_tensor_tensor(
                ztsb[:, h, :], zw[:, h, :], 1.0 / WL_SCALE, ztsb[:, h, :],
                op0=ALU.mult, op1=ALU.add,
            )
        for j in range(gt // 128):
            t0 = g * GT + off + j * 128
            _routing_tile(
                nc, tc, ztsb, j, t0, bias_sb, ident, zf_pool, r_pool, sm_pool,
                wout_d, iout_d,
            )

    ctx.close()


def _routing_tile(
    nc, tc, ztsb, j, t0, bias_sb, ident, zf_pool, r_pool, sm_pool, wout_d, iout_d
):
    # transpose z^T block back to [tok, e] and apply sigmoid
    zf = zf_pool.tile([128, 2, 128], F32, tag="zf")
    scores = r_pool.tile([128, E], F32, tag="scores")
    for h in range(2):
        nc.tensor.transpose(zf[:, h, :], ztsb[:, h, j * 128 : (j + 1) * 128], ident)
        nc.scalar.activation(
            scores[:, h * 128 : (h + 1) * 128], zf[:, h, :], ACTF.Sigmoid
        )

    # s = scores + bias (selection key)
    s = r_pool.tile([128, E], F32, tag="s")
    nc.vector.tensor_tensor(s, scores, bias_sb, op=ALU.add)

    # group scores: sum of top-2 within each group of 32
    gtop = sm_pool.tile([128, N_GROUPS, 8], F32, tag="gtop")
    for grp in range(N_GROUPS):
        nc.vector.max(gtop[:, grp, :], s[:, grp * EPG : (grp + 1) * EPG])
    gscore = sm_pool.tile([128, N_GROUPS], F32, tag="gscore")
    nc.vector.tensor_tensor(gscore, gtop[:, :, 0], gtop[:, :, 1], op=ALU.add)

    # top-4 groups: sort the 8 group scores, threshold at the 4th
    gsort = sm_pool.tile([128, 8], F32, tag="gsort")
    nc.vector.max(gsort, gscore)
    keep = sm_pool.tile([128, N_GROUPS], F32, tag="keep")
    nc.vector.tensor_scalar(
        keep, gscore, gsort[:, TOPK_GROUPS - 1 : TOPK_GROUPS], None, op0=ALU.is_ge
    )

    # masked selection key: s * keep (keep broadcast over each group of 32)
    sm_t = r_pool.tile([128, E], F32, tag="smask")
    nc.vector.tensor_tensor(
        sm_t.rearrange("p (g e) -> p g e", g=N_GROUPS),
        s.rearrange("p (g e) -> p g e", g=N_GROUPS),
        keep[:, :, None].broadcast_to([128, N_GROUPS, EPG]),
        op=ALU.mult,
    )

    # top-8 experts by masked biased score
    v8 = sm_pool.tile([128, 8], F32, tag="v8")
    nc.vector.max(v8, sm_t)
    idx8 = sm_pool.tile([128, 8], U32, tag="idx8")
    nc.vector.max_index(idx8, v8, sm_t)

    # indicator of the selected 8 positions (kill them, then compare)
    srest = r_pool.tile([128, E], F32, tag="srest")
    nc.vector.match_replace(
        out=srest, in_to_replace=v8, in_values=sm_t, imm_value=-1e30
    )
    ind = r_pool.tile([128, E], F32, tag="ind")
    nc.vector.tensor_scalar(ind, srest, -1e29, None, op0=ALU.is_le)

    # selected raw scores (zeros elsewhere)
    scsel = r_pool.tile([128, E], F32, tag="scsel")
    nc.vector.tensor_tensor(scsel, scores, ind, op=ALU.mult)

    # the 8 selected scores, sorted by score (order differs from idx8's order)
    s8 = sm_pool.tile([128, 8], F32, tag="s8")
    nc.vector.max(s8, scsel)
    sidx8 = sm_pool.tile([128, 8], U32, tag="sidx8")
    nc.vector.max_index(sidx8, s8, scsel)

    # sum of the selected scores (+eps)
    sumw = sm_pool.tile([128, 1], F32, tag="sumw")
    nc.vector.reduce_sum(sumw, s8, axis=bass_rust.AxisListType.X)
    nc.vector.tensor_scalar_add(sumw, sumw, 1e-20)

    # reorder s8 into idx8's (selection) order via an 8x8 match matrix:
    # w[k] = sum_j (idx8[k]==sidx8[j]) * s8[j]
    idx8f = sm_pool.tile([128, 8], F32, tag="idx8f")
    nc.vector.tensor_copy(idx8f, idx8)
    sidx8f = sm_pool.tile([128, 8], F32, tag="sidx8f")
    nc.vector.tensor_copy(sidx8f, sidx8)
    eq = sm_pool.tile([128, 8, 8], F32, tag="eq")
    nc.vector.tensor_tensor(
        eq,
        idx8f[:, :, None].broadcast_to([128, 8, 8]),
        sidx8f[:, None, :].broadcast_to([128, 8, 8]),
        op=ALU.is_equal,
    )
    nc.vector.tensor_tensor(
        eq, eq, s8[:, None, :].broadcast_to([128, 8, 8]), op=ALU.mult
    )
    wacc = sm_pool.tile([128, 8], F32, tag="wacc")
    nc.vector.tensor_reduce(wacc, eq, axis=bass_rust.AxisListType.X, op=ALU.add)

    # normalize + scale
    winv = sm_pool.tile([128, 1], F32, tag="winv")
    nc.vector.reciprocal(winv, sumw)
    wout = sm_pool.tile([128, 8], F32, tag="wout")
    nc.vector.tensor_scalar(wout, wacc, winv[:, 0:1], SCALE, op0=ALU.mult, op1=ALU.mult)
    iout = sm_pool.tile([128, 8], I32, tag="iout")
    nc.vector.tensor_copy(iout, idx8)

    nc.scalar.dma_start(wout_d[t0 : t0 + 128, :], wout)
    nc.scalar.dma_start(iout_d[t0 : t0 + 128, :], iout)


def make_nc():
    nc = bacc.Bacc(
        "TRN2",
        target_bir_lowering=False,
        debug=False,
        enable_asserts=False,
        num_devices=N_CORES,
    )
    aps = {
        "xh": nc.dram_tensor("xh", [NG, 128, NK, GT], F16, kind="ExternalInput").ap(),
        "xl": nc.dram_tensor("xl", [NG, 128, NK, GT], F16, kind="ExternalInput").ap(),
        "wh": nc.dram_tensor("wh", [128, NK, E], F16, kind="ExternalInput").ap(),
        "wl": nc.dram_tensor("wl", [128, NK, E], F16, kind="ExternalInput").ap(),
        "b": nc.dram_tensor("b", [E], F32, kind="ExternalInput").ap(),
        "w_out": nc.dram_tensor(
            "w_out", [T_CORE, TOP_K], F32, kind="ExternalOutput"
        ).ap(),
        "i_out": nc.dram_tensor(
            "i_out", [T_CORE, TOP_K], I32, kind="ExternalOutput"
        ).ap(),
    }
    with tile.TileContext(nc) as tc:
        build(tc, aps)
    nc.compile()
    return nc


_CACHED = {}


def _get_nc():
    if "nc" not in _CACHED:
        _CACHED["nc"] = make_nc()
    return _CACHED["nc"]


def _split_f16(a32):
    """fp16 hi/lo split, identical rounding to the on-chip ACT copy + DVE
    subtract the v1 kernel used: hi = rne_f16(a); lo = rne_f16(a - hi)."""
    hi = a32.astype(np.float16)
    lo = (a32 - hi.astype(np.float32)).astype(np.float16)
    return hi, lo


def kernel(x_TD, kernel_DE, bias_E, profile=False, trace_kwargs=None):
    x_TD = np.asarray(x_TD, dtype=np.float32)
    kernel_DE = np.asarray(kernel_DE, dtype=np.float32)
    bias_E = np.ascontiguousarray(np.asarray(bias_E, dtype=np.float32))
    assert x_TD.shape == (T_FULL, D_FULL)

    # host-side hi/lo split + transpose into the DMA-friendly layout
    xh, xl = _split_f16(x_TD)
    wh = kernel_DE.astype(np.float16)
    wl = ((kernel_DE - wh.astype(np.float32)) * WL_SCALE).astype(np.float16)

    # x: [T, D] -> per core [NG, 128(p), NK(c), GT(t)];  tok = g*GT+t, d = c*128+p
    def xr(a):
        a = a.reshape(N_CORES, NG, GT, NK, 128)
        return np.ascontiguousarray(a.transpose(0, 1, 4, 3, 2))

    xh_r = xr(xh)
    xl_r = xr(xl)
    # W: [D, E] -> [128(p), NK(c), E]
    wh_r = np.ascontiguousarray(wh.reshape(NK, 128, E).transpose(1, 0, 2))
    wl_r = np.ascontiguousarray(wl.reshape(NK, 128, E).transpose(1, 0, 2))

    nc = _get_nc()
    in_maps = [
        {
            "xh": xh_r[i],
            "xl": xl_r[i],
            "wh": wh_r,
            "wl": wl_r,
            "b": bias_E,
        }
        for i in range(N_CORES)
    ]
    res = bass_utils.run_bass_kernel_spmd(
        nc,
        in_maps,
        core_ids=list(range(N_CORES)),
        trace=profile,
        **(trace_kwargs or {}),
    )
    w_full = np.concatenate([res.results[i]["w_out"] for i in range(N_CORES)], axis=0)
    i_full = np.concatenate([res.results[i]["i_out"] for i in range(N_CORES)], axis=0)
    i_full = i_full.astype(np.int32)
    if profile:
        return (w_full, i_full), res
    return w_full, i_full
